# revision 1
# baseline (speedup 1.0000x reference)
"""Linear attention (silu+1 feature map) MultiHeadAttention kernel for 8x TRN2.

Sharding: data-parallel over batch (B=8 -> 1 batch element per NeuronCore).
Per-core math (T=4096, D=1024, H=16, Dh=64), all matmuls bf16 / fp32 PSUM:

  phase 1 (stream token tiles):
    qT[o,t]   = WqT.T @ xT          (feature-major, stationary = WqT chunks)
    phi_qT    = silu(s*qT + s*bq) + 1        (stored bf16, feature-major)
    k[t,e]    = xT.T @ WkT          (token-major, stationary = xT chunks)
    v[t,e]    = xT.T @ WvT + bv
    phi_k     = silu(s*k) + 1
    vk_h[e,d] += v_h.T @ phi_k_h    (PSUM accumulate, 16 heads packed in 1 bank)
  M stage:
    M_h[d,o]  = vk_h.T @ WoT_h      ->  M = vstack_h(M_h)   [1024,1024] bf16
  phase 2:
    yT[o,t]   = M.T @ phi_qT + bo   (one dense GEMM; folds per-head phi_q@kv
                                     and the output projection together)

Host side: transposes x per batch, pre-transposes/casts weights to bf16,
gathers yT.T per core. Output fp32.
"""

import numpy as np
import ml_dtypes

B, T, D = 8, 4096, 1024
H, DH = 16, 64
SCALE = float(DH ** -0.25)
NCORES = 8
P = 128
DC = D // P          # 8 feature chunks
TT = 512             # token tile (phase 1)
NTT = T // TT        # 8 token tiles
NSUB = TT // P       # 4 sub-tiles of 128 tokens

_BF16 = ml_dtypes.bfloat16

_CACHE = {}


def _split_multi_waits(nc):
    """walrus in this container only encodes ONE sync-wait command per
    instruction. Hoist extra waits onto injected same-engine NOPs placed
    immediately before the instruction (program order on the engine queue
    makes this semantically identical)."""
    import concourse.mybir as mybir

    n_split = 0
    for fn in nc.m.functions:
        for bb in fn.blocks:
            new = []
            changed = False
            for inst in bb.instructions:
                si = inst.sync_info
                waits = list(si.on_wait) if si is not None else []
                if len(waits) > 1:
                    changed = True
                    for j, w in enumerate(waits[:-1]):
                        nop = mybir.InstNoOp(
                            name=f"{inst.name}-sw{j}", ins=[], outs=[]
                        )
                        nop.engine = inst.engine
                        nop.sync_info = mybir.SyncInfo(
                            on_wait=[w], on_update=[]
                        )
                        new.append(nop)
                        n_split += 1
                    inst.sync_info = mybir.SyncInfo(
                        on_wait=[waits[-1]], on_update=list(si.on_update)
                    )
                new.append(inst)
            if changed:
                bb.instructions = new
    return n_split


def _build_program(debug=False):
    import concourse.bass as bass
    import concourse.mybir as mybir
    from concourse.tile import TileContext, add_dep_helper

    dt = mybir.dt
    AF = mybir.ActivationFunctionType

    nc = bass.Bass()

    xT_d = nc.dram_tensor("xT", [D, T], dt.bfloat16, kind="ExternalInput")
    wq_d = nc.dram_tensor("wq", [D, D], dt.bfloat16, kind="ExternalInput")
    wk_d = nc.dram_tensor("wk", [D, D], dt.bfloat16, kind="ExternalInput")
    wv_d = nc.dram_tensor("wv", [D, D], dt.bfloat16, kind="ExternalInput")
    wo_d = nc.dram_tensor("wo", [D, D], dt.bfloat16, kind="ExternalInput")
    bqs_d = nc.dram_tensor("bqs", [P, DC], dt.float32, kind="ExternalInput")
    bos_d = nc.dram_tensor("bos", [P, DC], dt.float32, kind="ExternalInput")
    bvb_d = nc.dram_tensor("bvb", [P, D], dt.float32, kind="ExternalInput")
    yT_d = nc.dram_tensor("yT", [D, T], dt.float32, kind="ExternalOutput")
    if debug:
        phiq_d = nc.dram_tensor("phiq_dump", [P, DC, T], dt.bfloat16, kind="ExternalOutput")
        kv_d = nc.dram_tensor("kv_dump", [P, 512], dt.float32, kind="ExternalOutput")
        m_d = nc.dram_tensor("m_dump", [P, DC, D], dt.bfloat16, kind="ExternalOutput")
        kproj_d = nc.dram_tensor("kproj_dump", [P, D], dt.float32, kind="ExternalOutput")
        vproj_d = nc.dram_tensor("vproj_dump", [P, D], dt.float32, kind="ExternalOutput")

    with TileContext(nc) as tc:
        with (
            tc.tile_pool(name="weights", bufs=1) as wpool,
            tc.tile_pool(name="phiq", bufs=1) as qpool,
            tc.tile_pool(name="msb", bufs=1) as mpool,
            tc.tile_pool(name="xin", bufs=3) as xpool,
            tc.tile_pool(name="kvtiles", bufs=6) as kvpool,
            tc.tile_pool(name="yout", bufs=2) as ypool,
        ):
            # ---- weight / const preload ----
            # wq + the first x pair come first (they gate the first matmuls);
            # x tiles stream on the gpsimd queue, weights on sync, wo (only
            # needed at the M stage) last.
            wq_sb = wpool.tile([P, DC, D], dt.bfloat16, tag="wq")
            wk_sb = wpool.tile([P, DC, D], dt.bfloat16, tag="wk")
            wv_sb = wpool.tile([P, DC, D], dt.bfloat16, tag="wv")
            wo_sb = wpool.tile([P, DC, D], dt.bfloat16, tag="wo")
            bq_sb = wpool.tile([P, DC], dt.float32, tag="bq")
            bo_sb = wpool.tile([P, DC], dt.float32, tag="bo")
            bv_sb = wpool.tile([P, D], dt.float32, tag="bv")
            nc.sync.dma_start(bq_sb[:], bqs_d[:])
            nc.sync.dma_start(bo_sb[:], bos_d[:])
            # wq in column halves: the first q matmuls (oc 0-3) only need the
            # first half, so PE starts ~3us earlier. bvb (0.5MB) is not needed
            # until the first kv sub-tile (~30us in), so it loads after wk.
            wq_r = wq_d.rearrange("(c p) o -> p c o", p=P)
            xT_r = xT_d.rearrange("(c p) t -> p c t", p=P)
            nc.sync.dma_start(wq_sb[:, :, 0:512], wq_r[:, :, 0:512])
            xt_pre = []
            for half in range(2):
                xt0 = xpool.tile([P, DC, TT], dt.bfloat16, tag="xt", name=f"xtpre{half}")
                nc.sync.dma_start(xt0[:], xT_r[:, :, half * TT : (half + 1) * TT])
                xt_pre.append(xt0)
            nc.sync.dma_start(wq_sb[:, :, 512:1024], wq_r[:, :, 512:1024])
            nc.sync.dma_start(wk_sb[:], wk_d.rearrange("(c p) o -> p c o", p=P))
            nc.sync.dma_start(bv_sb[:], bvb_d[:])
            nc.sync.dma_start(wv_sb[:], wv_d.rearrange("(c p) o -> p c o", p=P))
            nc.sync.dma_start(wo_sb[:], wo_d.rearrange("(c p) o -> p c o", p=P))

            phi_q = qpool.tile([P, DC, T], dt.bfloat16, tag="phiq")
            m_chunks = []
            for c in range(DC):
                m_chunk = mpool.tile(
                    [P, D], dt.bfloat16, tag=f"msb{c}", name=f"msb{c}"
                )
                m_chunks.append(m_chunk)
            kv_chunks = []
            for c in range(DC):
                kvc = mpool.tile(
                    [P, P], dt.bfloat16, tag=f"kvsb{c}", name=f"kvsb{c}"
                )
                kv_chunks.append(kvc)

            zz = wpool.tile([1, 640], dt.bfloat16, tag="zz")
            nc.vector.memset(zz[:], 0.0)
            # kv chunk off-diagonal blocks must be zero (block-diag repack)
            for c in range(DC):
                nc.vector.memset(kv_chunks[c][:], 0.0)

            with tc.tile_pool(name="ps_kv", bufs=1, space="PSUM") as pkv_pool:
                kv_ps = pkv_pool.tile([P, 512], dt.float32, tag="kvacc")
                # zero the whole kv bank once (sets has_written for every
                # element) so the 16 interleaved head slots can accumulate
                # with start=False; multiple start=True groups in one bank
                # clobber each other.
                nc.tensor.matmul(
                    kv_ps[:], lhsT=zz[:1, :P], rhs=zz[:1, P : P + 512],
                    start=True, stop=True, skip_group_check=True,
                )
                # warmup matmuls filling the startup DMA shadow: semantically
                # they re-write zeros over the (unused-yet) kv bank, but they
                # keep the PE p-state/HAM warm so the first real matmuls run
                # at full clock the moment weights land. N=128 keeps the
                # granularity fine so the last one barely delays real work.
                for w in range(88):
                    nc.tensor.matmul(
                        kv_ps[:, 0:128], lhsT=zz[:1, :P], rhs=zz[:1, P : P + 128],
                        start=True, stop=True, skip_group_check=True,
                    )
                nc.tensor.matmul(
                    kv_ps[:], lhsT=zz[:1, :P], rhs=zz[:1, P : P + 512],
                    start=True, stop=True, skip_group_check=True,
                )

                kv_pend = [None]

                def _emit_kv(pending, last):
                    phik_p, vsb_p = pending
                    for h in range(H):
                        r0 = (h % 2) * 64
                        c0 = (h // 2) * 64
                        nc.tensor.matmul(
                            kv_ps[r0 : r0 + 64, c0 : c0 + 64],
                            lhsT=vsb_p[:, h * 64 : (h + 1) * 64],
                            rhs=phik_p[:, h * 64 : (h + 1) * 64],
                            start=False,
                            stop=last and h == H - 1,
                            skip_group_check=True,
                        )

                with tc.tile_pool(name="ps_q", bufs=3, space="PSUM") as pq_pool:
                  with tc.tile_pool(name="ps_kvp", bufs=2, space="PSUM") as pkvp_pool:
                      # ---- q projection (feature-major out); both tiles of a
                      # pair share each stationary weight load. first_split runs
                      # tile A before tile B (pair 0: B's DMA still in flight).
                      # flush_after_oc0 emits the last kv matmuls between q
                      # chunks so the M stage overlaps the q stream.
                      def _q_section(pair, xts, first_split, post_oc=None):
                          def _drain(oc, half, psx):
                              tt = pair * 2 + half
                              pq_slice = phi_q[:, oc, tt * TT : (tt + 1) * TT]
                              nc.scalar.activation(
                                  pq_slice, psx[:], AF.Silu,
                                  bias=bq_sb[:, oc : oc + 1], scale=SCALE,
                              )
                              nc.vector.tensor_scalar_add(pq_slice, pq_slice, 1.0)

                          for oc in range(DC):
                              psA = pq_pool.tile([P, TT], dt.float32, tag="psq")
                              psB = pq_pool.tile([P, TT], dt.float32, tag="psq")
                              if first_split and oc == 0:
                                  last_a = None
                                  for d in range(DC):
                                      last_a = nc.tensor.matmul(
                                          psA[:],
                                          lhsT=wq_sb[:, d, oc * P : (oc + 1) * P],
                                          rhs=xts[0][:, d, :],
                                          start=(d == 0),
                                          stop=(d == DC - 1),
                                      )
                                  # bridge the B-tile DMA wait with warmup
                                  # zero-rewrites of the (still unused) kv bank;
                                  # dep-pinned after the A matmuls so the
                                  # scheduler cannot hoist them earlier
                                  for w in range(14):
                                      dmy = nc.tensor.matmul(
                                          kv_ps[:, 0:128],
                                          lhsT=zz[:1, :P],
                                          rhs=zz[:1, P : P + 128],
                                          start=True, stop=True,
                                          skip_group_check=True,
                                      )
                                      add_dep_helper(
                                          dmy.ins, last_a.ins, sync=False,
                                          reason="bridge dummies after A matmuls",
                                      )
                                  dmy = nc.tensor.matmul(
                                      kv_ps[:], lhsT=zz[:1, :P],
                                      rhs=zz[:1, P : P + 512],
                                      start=True, stop=True,
                                      skip_group_check=True,
                                  )
                                  add_dep_helper(
                                      dmy.ins, last_a.ins, sync=False,
                                      reason="bridge dummies after A matmuls",
                                  )
                                  for d in range(DC):
                                      nc.tensor.matmul(
                                          psB[:],
                                          lhsT=wq_sb[:, d, oc * P : (oc + 1) * P],
                                          rhs=xts[1][:, d, :],
                                          start=(d == 0),
                                          stop=(d == DC - 1),
                                      )
                              else:
                                  for d in range(DC):
                                      nc.tensor.matmul(
                                          psA[:],
                                          lhsT=wq_sb[:, d, oc * P : (oc + 1) * P],
                                          rhs=xts[0][:, d, :],
                                          start=(d == 0),
                                          stop=(d == DC - 1),
                                      )
                                      nc.tensor.matmul(
                                          psB[:],
                                          lhsT=wq_sb[:, d, oc * P : (oc + 1) * P],
                                          rhs=xts[1][:, d, :],
                                          start=(d == 0),
                                          stop=(d == DC - 1),
                                      )
                              _drain(oc, 0, psA)
                              _drain(oc, 1, psB)
                              if post_oc is not None and oc in post_oc:
                                  post_oc[oc]()

                      # ---- k,v projections (token-major) + kv accumulation.
                      # The 16 kv-accumulate matmuls for a sub-tile are emitted
                      # one sub-tile LATE so PE never waits on silu/+bv. ----
                      def _kvproj_section(pair, xts):
                          for half in range(2):
                              tt = pair * 2 + half
                              xt = xts[half]
                              for sub in range(NSUB):
                                  pk = pkvp_pool.tile([P, D], dt.float32, tag="pkv")
                                  pv = pkvp_pool.tile([P, D], dt.float32, tag="pkv")
                                  xs = xt[:, :, sub * P : (sub + 1) * P]
                                  for d in range(DC):
                                      for n in range(2):
                                          nc.tensor.matmul(
                                              pk[:, n * 512 : (n + 1) * 512],
                                              lhsT=xs[:, d, :],
                                              rhs=wk_sb[:, d, n * 512 : (n + 1) * 512],
                                              start=(d == 0),
                                              stop=(d == DC - 1),
                                          )
                                      for n in range(2):
                                          nc.tensor.matmul(
                                              pv[:, n * 512 : (n + 1) * 512],
                                              lhsT=xs[:, d, :],
                                              rhs=wv_sb[:, d, n * 512 : (n + 1) * 512],
                                              start=(d == 0),
                                              stop=(d == DC - 1),
                                          )
                                  if debug and tt == 0 and sub == 0:
                                      kpf = mpool.tile([P, D], dt.float32, tag="kpdump")
                                      vpf = mpool.tile([P, D], dt.float32, tag="vpdump")
                                      nc.vector.tensor_copy(out=kpf[:], in_=pk[:])
                                      nc.vector.tensor_copy(out=vpf[:], in_=pv[:])
                                      nc.sync.dma_start(kproj_d[:], kpf[:])
                                      nc.sync.dma_start(vproj_d[:], vpf[:])
                                  phik = kvpool.tile([P, D], dt.bfloat16, tag="phik")
                                  vsb = kvpool.tile([P, D], dt.bfloat16, tag="vsb")
                                  nc.scalar.activation(
                                      phik[:], pk[:], AF.Silu, scale=SCALE
                                  )
                                  nc.vector.tensor_scalar_add(phik[:], phik[:], 1.0)
                                  nc.vector.tensor_add(vsb[:], pv[:], bv_sb[:])
                                  if kv_pend[0] is not None:
                                      _emit_kv(kv_pend[0], False)
                                  kv_pend[0] = (phik, vsb)

                      for pair in range(NTT // 2):
                          if pair == 0:
                              xts = xt_pre
                          else:
                              xts = []
                              for half in range(2):
                                  tt = pair * 2 + half
                                  xt = xpool.tile([P, DC, TT], dt.bfloat16, tag="xt")
                                  nc.gpsimd.dma_start(
                                      xt[:], xT_r[:, :, tt * TT : (tt + 1) * TT]
                                  )
                                  xts.append(xt)

                          if pair == NTT // 2 - 1:
                              # last pair: kvproj first, then q. The kv flush,
                              # repack copies, M matmuls (psum borrowed from the
                              # drained kvproj pool — no extra banks) and
                              # m-chunk copies are spread across the q chunk
                              # boundaries, fully hidden under the 27us of q
                              # matmuls with no engine head-of-line blocking.
                              _kvproj_section(pair, xts)

                              def _hook_flush():
                                  _emit_kv(kv_pend[0], True)
                                  kv_pend[0] = None
                                  for c in range(DC):
                                      if c % 2 == 0:
                                          nc.vector.tensor_copy(
                                              out=kv_chunks[c][0:64, 0:64],
                                              in_=kv_ps[0:64, c * 64 : (c + 1) * 64],
                                          )
                                          nc.vector.tensor_copy(
                                              out=kv_chunks[c][64:128, 64:128],
                                              in_=kv_ps[64:128, c * 64 : (c + 1) * 64],
                                          )
                                      else:
                                          nc.scalar.copy(
                                              out=kv_chunks[c][0:64, 0:64],
                                              in_=kv_ps[0:64, c * 64 : (c + 1) * 64],
                                          )
                                          nc.scalar.copy(
                                              out=kv_chunks[c][64:128, 64:128],
                                              in_=kv_ps[64:128, c * 64 : (c + 1) * 64],
                                          )

                              def _mk_hook_m(c0):
                                  def _hook():
                                      for c in (c0, c0 + 1):
                                          pm = pkvp_pool.tile(
                                              [P, D], dt.float32, tag="pkv"
                                          )
                                          for n in range(2):
                                              nc.tensor.matmul(
                                                  pm[:, n * 512 : (n + 1) * 512],
                                                  lhsT=kv_chunks[c][:],
                                                  rhs=wo_sb[:, c, n * 512 : (n + 1) * 512],
                                                  start=True,
                                                  stop=True,
                                              )
                                          nc.vector.tensor_copy(
                                              out=m_chunks[c][:, 0:512],
                                              in_=pm[:, 0:512],
                                          )
                                          nc.scalar.copy(
                                              out=m_chunks[c][:, 512:1024],
                                              in_=pm[:, 512:1024],
                                          )
                                  return _hook

                              hooks = {0: _hook_flush}
                              for c0 in range(0, DC, 2):
                                  hooks[1 + c0 // 2] = _mk_hook_m(c0)
                              _q_section(pair, xts, False, hooks)
                          else:
                              _q_section(pair, xts, pair == 0)
                              _kvproj_section(pair, xts)

                      if kv_pend[0] is not None:
                          _emit_kv(kv_pend[0], True)
                          kv_pend[0] = None

                  if debug:
                      kvf = mpool.tile([P, 512], dt.float32, tag="kvdump")
                      nc.vector.tensor_copy(out=kvf[:], in_=kv_ps[:])
                      nc.sync.dma_start(kv_d[:], kvf[:])
                  # ---- phase 2: yT = M.T @ phi_q + bo ----
                  # [128,1024] psum tiles (bufs=4): the whole-tile RAW window is 16
                  # matmuls, so each tile's drain overlaps the next tiles' matmuls
                  # and the kernel tail is just one small tile's drain.
                  with tc.tile_pool(name="ps_y", bufs=2, space="PSUM") as py_pool:
                      for oc in range(DC):
                          for qb in range(4):
                              if oc == DC - 1 and qb == 3:
                                  # very last block: two independent [128,512] psum
                                  # tiles so the final drain is one small piece that
                                  # starts 8 matmuls before the end
                                  for i in range(2):
                                      pyf = py_pool.tile([P, 512], dt.float32, tag="py")
                                      for j in range(DC):
                                          f = (oc * 4 + qb + j) % DC
                                          nc.tensor.matmul(
                                              pyf[:],
                                              lhsT=m_chunks[f][:, oc * P : (oc + 1) * P],
                                              rhs=phi_q[
                                                  :, f, qb * 1024 + i * 512 : qb * 1024 + (i + 1) * 512
                                              ],
                                              start=(j == 0),
                                              stop=(j == DC - 1),
                                          )
                                      ysf = ypool.tile(
                                          [P, 512], dt.float32, tag=f"ys{i}"
                                      )
                                      if i == 0:
                                          nc.scalar.activation(
                                              ysf[:], pyf[:], AF.Identity,
                                              bias=bo_sb[:, oc : oc + 1], scale=1.0,
                                          )
                                          nc.sync.dma_start(
                                              yT_d[
                                                  oc * P : (oc + 1) * P,
                                                  qb * 1024 : qb * 1024 + 512,
                                              ],
                                              ysf[:],
                                          )
                                      else:
                                          nc.vector.tensor_scalar_add(
                                              ysf[:], pyf[:], bo_sb[:, oc : oc + 1]
                                          )
                                          nc.gpsimd.dma_start(
                                              yT_d[
                                                  oc * P : (oc + 1) * P,
                                                  qb * 1024 + 512 : (qb + 1) * 1024,
                                              ],
                                              ysf[:],
                                          )
                                  continue
                              py = py_pool.tile([P, 1024], dt.float32, tag="py")
                              # rotated f-order: successive tiles start on different
                              # M chunks, so phase 2 begins as soon as the first
                              # chunk copy lands and the rest overlap these matmuls
                              for j in range(DC):
                                  f = (oc * 4 + qb + j) % DC
                                  for i in range(2):
                                      nc.tensor.matmul(
                                          py[:, i * 512 : (i + 1) * 512],
                                          lhsT=m_chunks[f][:, oc * P : (oc + 1) * P],
                                          rhs=phi_q[
                                              :, f, qb * 1024 + i * 512 : qb * 1024 + (i + 1) * 512
                                          ],
                                          start=(j == 0),
                                          stop=(j == DC - 1),
                                      )
                              # drain in two 512 pieces on ACT+sync / DVE+gpsimd
                              ys0 = ypool.tile([P, 512], dt.float32, tag="ys0")
                              nc.scalar.activation(
                                  ys0[:], py[:, 0:512],
                                  AF.Identity, bias=bo_sb[:, oc : oc + 1], scale=1.0,
                              )
                              nc.sync.dma_start(
                                  yT_d[
                                      oc * P : (oc + 1) * P,
                                      qb * 1024 : qb * 1024 + 512,
                                  ],
                                  ys0[:],
                              )
                              ys1 = ypool.tile([P, 512], dt.float32, tag="ys1")
                              nc.vector.tensor_scalar_add(
                                  ys1[:], py[:, 512:1024], bo_sb[:, oc : oc + 1]
                              )
                              nc.gpsimd.dma_start(
                                  yT_d[
                                      oc * P : (oc + 1) * P,
                                      qb * 1024 + 512 : (qb + 1) * 1024,
                                  ],
                                  ys1[:],
                              )

            if debug:
                nc.sync.dma_start(phiq_d[:], phi_q[:])
                for c in range(DC):
                    nc.sync.dma_start(m_d[:, c, :], m_chunks[c][:])
    _split_multi_waits(nc)
    return nc


def _get_program(debug=False):
    key = ("nc", debug)
    if key not in _CACHE:
        _CACHE[key] = _build_program(debug)
    return _CACHE[key]


def _prep_shared(Wq, bq, Wk, Wv, bv, Wo, bo):
    shared = {
        "wq": np.ascontiguousarray(Wq.T).astype(_BF16),
        "wk": np.ascontiguousarray(Wk.T).astype(_BF16),
        "wv": np.ascontiguousarray(Wv.T).astype(_BF16),
        "wo": np.ascontiguousarray(Wo.T).astype(_BF16),
        "bqs": np.ascontiguousarray(
            (SCALE * bq).astype(np.float32).reshape(DC, P).T
        ),
        "bos": np.ascontiguousarray(bo.astype(np.float32).reshape(DC, P).T),
        "bvb": np.ascontiguousarray(
            np.broadcast_to(bv.astype(np.float32), (P, D))
        ),
    }
    return shared


def _run(in_maps, trace=False, debug=False, **kw):
    from concourse.bass_utils import run_bass_kernel_spmd

    nc = _get_program(debug)
    return run_bass_kernel_spmd(nc, in_maps, list(range(NCORES)), trace=trace, **kw)


def kernel(x, Wq, bq, Wk, Wv, bv, Wo, bo):
    x = np.asarray(x, dtype=np.float32)
    assert x.shape == (B, T, D), x.shape
    shared = _prep_shared(
        np.asarray(Wq, np.float32), np.asarray(bq, np.float32),
        np.asarray(Wk, np.float32), np.asarray(Wv, np.float32),
        np.asarray(bv, np.float32), np.asarray(Wo, np.float32),
        np.asarray(bo, np.float32),
    )
    in_maps = []
    for b in range(B):
        m = dict(shared)
        m["xT"] = np.ascontiguousarray(x[b].T).astype(_BF16)
        in_maps.append(m)

    res = _run(in_maps)
    out = np.empty((B, T, D), np.float32)
    for b in range(B):
        out[b] = res.results[b]["yT"].T
    return out



# revision 8
# speedup vs baseline: 3.1027x; 3.1027x over previous
"""Linear attention (silu+1 feature map) MultiHeadAttention kernel for 8x TRN2.

Sharding: data-parallel over batch (B=8 -> 1 batch element per NeuronCore).

fp8 DoubleRow design. All four big GEMMs run as fp8e4 DoubleRow matmuls
(contraction 256 per instruction at 0.5 cycles/output-row). Accuracy is
preserved by a mean-split: phi(x) = silu(x)+1 = 1 + d(x), so every fp8
operand that multiplies a full-magnitude tensor is a small delta:

  phase 1 (token tiles of 512, processed in pairs):
    qT[o,t]  = wq8.T @ x8T            (DoubleRow, feature-major out)
    dq[o,t]  = silu(s*qT + s*bq)      (ACT, fp8 out; phi_q = 1 + dq)
    k[t,o]   = x8T.T @ wk8            (DoubleRow, token-major)
    v[t,e]   = x8T.T @ wv8 + bv       (DoubleRow + DVE bias add, fp8 out)
    dk[t,o]  = silu(s*k)              (ACT, fp8 out; phi_k = 1 + dk)
    kv[e,d]_h += v_h.T @ dk_h         (DoubleRow over token sub-tile pairs,
                                       [64,1024] psum: head h at cols 64h)
  corrections (the "1" parts of phi_k):
    kv_full  = kv + S_v[e]            S_v = sum_t v_exact[t] computed on host
                                      from the original fp32 x (rank-1 term)
  M stage:
    kv8dr    = block-diag fp8 repack of kv_full  ([64, 2, 128] per chunk)
    M[d,o]_c = kv8dr_c .T@ wo28_pair  (DoubleRow, contraction = 2 heads' e)
    m8       = fp8(M)
  colsum (the "1" part of phi_q): y += colsum(M) as a per-partition bias:
    rowsum_kv[e] = reduce_d kv_full   (DVE reduce + corr64)
    bias[o]  = sum_e rowsum_kv[e] wo[e,o] + bo   (128 N=1 matvecs, bf16)
  phase 2:
    yT[o,t]  = m8.T @ dq + bias       (DoubleRow; bias folded into drains)

Host side: fp8 casts (clip +-240), S_v correction, output upcast bf16->fp32.
"""

import numpy as np
import ml_dtypes

B, T, D = 8, 4096, 1024
H, DH = 16, 64
SCALE = float(DH ** -0.25)
NCORES = 8
P = 128
DC = D // P          # 8 feature chunks
NKP = DC // 2        # 4 k-pairs for DoubleRow
TT = 512             # token tile (phase 1)
NTT = T // TT        # 8 token tiles
NSUB = TT // P       # 4 sub-tiles of 128 tokens

_BF16 = ml_dtypes.bfloat16
_F8 = ml_dtypes.float8_e4m3fn

_CACHE = {}


def _split_multi_waits(nc):
    """walrus in this container only encodes ONE sync-wait command per
    instruction. Hoist extra waits onto injected same-engine NOPs placed
    immediately before the instruction (program order on the engine queue
    makes this semantically identical)."""
    import concourse.mybir as mybir

    n_split = 0
    for fn in nc.m.functions:
        for bb in fn.blocks:
            new = []
            changed = False
            for inst in bb.instructions:
                si = inst.sync_info
                waits = list(si.on_wait) if si is not None else []
                if len(waits) > 1:
                    changed = True
                    for j, w in enumerate(waits[:-1]):
                        nop = mybir.InstNoOp(
                            name=f"{inst.name}-sw{j}", ins=[], outs=[]
                        )
                        nop.engine = inst.engine
                        nop.sync_info = mybir.SyncInfo(
                            on_wait=[w], on_update=[]
                        )
                        new.append(nop)
                        n_split += 1
                    inst.sync_info = mybir.SyncInfo(
                        on_wait=[waits[-1]], on_update=list(si.on_update)
                    )
                new.append(inst)
            if changed:
                bb.instructions = new
    return n_split


def _build_program():
    import concourse.bass as bass
    import concourse.mybir as mybir
    from concourse.tile import TileContext, add_dep_helper

    dt = mybir.dt
    AF = mybir.ActivationFunctionType
    PM = mybir.MatmulPerfMode

    nc = bass.Bass()

    xT_d = nc.dram_tensor("xT", [D, T], dt.float8e4, kind="ExternalInput")
    wq_d = nc.dram_tensor("wq", [D, D], dt.float8e4, kind="ExternalInput")
    wk_d = nc.dram_tensor("wk", [D, D], dt.float8e4, kind="ExternalInput")
    wv_d = nc.dram_tensor("wv", [D, D], dt.float8e4, kind="ExternalInput")
    wo2_d = nc.dram_tensor("wo2", [64, H, D], dt.float8e4, kind="ExternalInput")
    wob_d = nc.dram_tensor("wob", [D, D], dt.bfloat16, kind="ExternalInput")
    bqs_d = nc.dram_tensor("bqs", [P, DC], dt.float32, kind="ExternalInput")
    bos_d = nc.dram_tensor("bos", [P, DC], dt.float32, kind="ExternalInput")
    bvb_d = nc.dram_tensor("bvb", [P, D], dt.float32, kind="ExternalInput")
    cor1_d = nc.dram_tensor("cor1", [64, H], dt.float32, kind="ExternalInput")
    cor64_d = nc.dram_tensor("cor64", [64, H], dt.float32, kind="ExternalInput")
    yT_d = nc.dram_tensor("yT", [D, T], dt.bfloat16, kind="ExternalOutput")

    with TileContext(nc) as tc:
        with (
            tc.tile_pool(name="weights", bufs=1) as wpool,
            tc.tile_pool(name="phiq", bufs=1) as qpool,
            tc.tile_pool(name="msb", bufs=1) as mpool,
            tc.tile_pool(name="xin", bufs=3) as xpool,
            tc.tile_pool(name="kvtiles", bufs=4) as kvpool,
            tc.tile_pool(name="yout", bufs=3) as ypool,
        ):
            # ---- weight / const preload ----
            wq_sb = wpool.tile([P, DC, D], dt.float8e4, tag="wq")
            wk_sb = wpool.tile([P, DC, D], dt.float8e4, tag="wk")
            wv_sb = wpool.tile([P, DC, D], dt.float8e4, tag="wv")
            wo2_sb = wpool.tile([64, H, D], dt.float8e4, tag="wo2")
            wob_sb = wpool.tile([P, DC, D], dt.bfloat16, tag="wob")
            bq_sb = wpool.tile([P, DC], dt.float32, tag="bq")
            bo_sb = wpool.tile([P, DC], dt.float32, tag="bo")
            bv_sb = wpool.tile([P, D], dt.float32, tag="bv")
            cor1_sb = wpool.tile([64, H], dt.float32, tag="cor1")
            cor64_sb = wpool.tile([64, H], dt.float32, tag="cor64")
            bias_sb = wpool.tile([P, DC], dt.float32, tag="bias")
            rsum_sb = wpool.tile([P, H], dt.bfloat16, tag="rsum")
            rsraw_sb = wpool.tile([64, H], dt.float32, tag="rsraw")

            wq_r = wq_d.rearrange("(c p) o -> p c o", p=P)
            xT_r = xT_d.rearrange("(c p) t -> p c t", p=P)

            # first q matmuls gate on the pair-0 x slab + first wq column
            # half (covers oc 0-3); x pairs 1-3 stream on the gpsimd queue.
            xt01 = xpool.tile([P, DC, 2 * TT], dt.float8e4, tag="xt0", bufs=1)
            nc.sync.dma_start(wq_sb[:, :, 0:512], wq_r[:, :, 0:512])
            nc.sync.dma_start(xt01[:], xT_r[:, :, 0 : 2 * TT])
            nc.sync.dma_start(wq_sb[:, :, 512:1024], wq_r[:, :, 512:1024])
            xt_pre = [xt01[:, :, 0:TT], xt01[:, :, TT : 2 * TT]]
            nc.sync.dma_start(bq_sb[:], bqs_d[:])
            nc.sync.dma_start(wk_sb[:], wk_d.rearrange("(c p) o -> p c o", p=P))
            nc.sync.dma_start(bv_sb[:], bvb_d[:])
            nc.sync.dma_start(wv_sb[:], wv_d.rearrange("(c p) o -> p c o", p=P))
            nc.sync.dma_start(bo_sb[:], bos_d[:])
            nc.sync.dma_start(cor1_sb[:], cor1_d[:])
            nc.sync.dma_start(cor64_sb[:], cor64_d[:])
            nc.sync.dma_start(wo2_sb[:], wo2_d[:])
            nc.sync.dma_start(wob_sb[:], wob_d.rearrange("(c p) o -> p c o", p=P))

            phi_q = qpool.tile([P, DC, T], dt.float8e4, tag="phiq")
            m8 = mpool.tile([P, DC, D], dt.float8e4, tag="m8")
            # block-diag fp8 repack of kv: chunk c = [64, 2, 128]; j=0 holds
            # head 2c in cols 0:64, j=1 holds head 2c+1 in cols 64:128, rest 0
            kv8 = mpool.tile([64, DC, 2, P], dt.float8e4, tag="kv8")

            zz = wpool.tile([1, 640], dt.bfloat16, tag="zz")
            nc.vector.memset(zz[:], 0.0)
            nc.gpsimd.memset(kv8[:], 0.0)

            with tc.tile_pool(name="ps_kv", bufs=1, space="PSUM") as pkv_pool:
                # kv accumulator: head h in columns [64h, 64h+64), rows = e
                kv_ps = pkv_pool.tile([64, H, 64], dt.float32, tag="kvacc")

                def _zero_kv(tag):
                    for i in range(2):
                        nc.tensor.matmul(
                            kv_ps[:, 8 * i : 8 * i + 8, :],
                            lhsT=zz[:1, :64], rhs=zz[:1, 64:576],
                            start=True, stop=True, skip_group_check=True,
                        )

                _zero_kv("z0")
                # warmup matmuls filling the startup DMA shadow: keep the PE
                # p-state warm so the first real matmuls run at full clock.
                for w in range(40):
                    nc.tensor.matmul(
                        kv_ps[:, 0:2, :], lhsT=zz[:1, :64], rhs=zz[:1, 64:192],
                        start=True, stop=True, skip_group_check=True,
                    )
                _zero_kv("z1")

                kv_pend = [None]

                def _emit_kv(pending, last):
                    v2_p, dk2_p = pending
                    for h in range(H):
                        nc.tensor.matmul(
                            kv_ps[:, h, :],
                            lhsT=v2_p[:, :, h * 64 : (h + 1) * 64],
                            rhs=dk2_p[:, :, h * 64 : (h + 1) * 64],
                            start=False,
                            stop=last and h == H - 1,
                            skip_group_check=True,
                            perf_mode=PM.DoubleRow,
                        )

                with tc.tile_pool(name="ps_big", bufs=3, space="PSUM") as pbig:

                    # ---- q projection (feature-major out). One [128,1024]
                    # psum per oc covers both tiles of the pair. ----
                    def _q_section(pair, xts, post_oc=None):
                        for oc in range(DC):
                            pq = pbig.tile([P, 2 * TT], dt.float32, tag="pbig")
                            for half in range(2):
                                for kp in range(NKP):
                                    nc.tensor.matmul(
                                        pq[:, half * TT : (half + 1) * TT],
                                        lhsT=wq_sb[:, 2 * kp : 2 * kp + 2, oc * P : (oc + 1) * P],
                                        rhs=xts[half][:, 2 * kp : 2 * kp + 2, :],
                                        start=(kp == 0),
                                        stop=(kp == NKP - 1),
                                        perf_mode=PM.DoubleRow,
                                    )
                            nc.scalar.activation(
                                phi_q[:, oc, pair * 2 * TT : (pair + 1) * 2 * TT],
                                pq[:], AF.Silu,
                                bias=bq_sb[:, oc : oc + 1], scale=SCALE,
                            )
                            if post_oc is not None and oc in post_oc:
                                post_oc[oc]()

                    # ---- k,v projections + kv accumulation (DoubleRow over
                    # sub-tile pairs; kv matmuls emitted one sub-pair late) ----
                    def _kv_section(pair, xts):
                        v2 = dk2 = None
                        for half in range(2):
                            xt = xts[half]
                            for sub in range(NSUB):
                                g = half * NSUB + sub
                                j = g % 2
                                xs = xt[:, :, sub * P : (sub + 1) * P]
                                pk = pbig.tile([P, D], dt.float32, tag="pbig")
                                pv = pbig.tile([P, D], dt.float32, tag="pbig")
                                for n in range(2):
                                    for kp in range(NKP):
                                        nc.tensor.matmul(
                                            pk[:, n * 512 : (n + 1) * 512],
                                            lhsT=xs[:, 2 * kp : 2 * kp + 2, :],
                                            rhs=wk_sb[:, 2 * kp : 2 * kp + 2, n * 512 : (n + 1) * 512],
                                            start=(kp == 0),
                                            stop=(kp == NKP - 1),
                                            perf_mode=PM.DoubleRow,
                                        )
                                for n in range(2):
                                    for kp in range(NKP):
                                        nc.tensor.matmul(
                                            pv[:, n * 512 : (n + 1) * 512],
                                            lhsT=xs[:, 2 * kp : 2 * kp + 2, :],
                                            rhs=wv_sb[:, 2 * kp : 2 * kp + 2, n * 512 : (n + 1) * 512],
                                            start=(kp == 0),
                                            stop=(kp == NKP - 1),
                                            perf_mode=PM.DoubleRow,
                                        )
                                if j == 0:
                                    v2 = kvpool.tile([P, 2, D], dt.float8e4, tag="v2")
                                    dk2 = kvpool.tile([P, 2, D], dt.float8e4, tag="dk2")
                                nc.scalar.activation(
                                    dk2[:, j, :], pk[:], AF.Silu, scale=SCALE
                                )
                                nc.vector.tensor_add(v2[:, j, :], pv[:], bv_sb[:])
                                if j == 1:
                                    if kv_pend[0] is not None:
                                        _emit_kv(kv_pend[0], False)
                                    kv_pend[0] = (v2, dk2)

                    for pair in range(NTT // 2):
                        if pair == 0:
                            xts = xt_pre
                        else:
                            xts = []
                            for half in range(2):
                                tt = pair * 2 + half
                                xt = xpool.tile([P, DC, TT], dt.float8e4, tag="xt")
                                nc.gpsimd.dma_start(
                                    xt[:], xT_r[:, :, tt * TT : (tt + 1) * TT]
                                )
                                xts.append(xt)

                        if pair == NTT // 2 - 1:
                            # last pair: kv section first; the kv flush,
                            # repack, M stage and colsum spread across the q
                            # chunk boundaries so they hide under q matmuls.
                            _kv_section(pair, xts)

                            def _hook_flush():
                                _emit_kv(kv_pend[0], True)
                                kv_pend[0] = None

                            def _hook_repack():
                                # kv can exceed fp8e4's +-240: store kv/8 in
                                # fp8 (wo2 is pre-scaled by 8 on the host)
                                for c in range(DC):
                                    nc.vector.tensor_scalar(
                                        kv8[:, c, 0, 0:64],
                                        kv_ps[:, 2 * c, :],
                                        cor1_sb[:, 2 * c : 2 * c + 1],
                                        0.125,
                                        op0=mybir.AluOpType.add,
                                        op1=mybir.AluOpType.mult,
                                    )
                                    nc.vector.tensor_scalar(
                                        kv8[:, c, 1, 64:128],
                                        kv_ps[:, 2 * c + 1, :],
                                        cor1_sb[:, 2 * c + 1 : 2 * c + 2],
                                        0.125,
                                        op0=mybir.AluOpType.add,
                                        op1=mybir.AluOpType.mult,
                                    )
                                nc.vector.tensor_reduce(
                                    rsraw_sb[:], kv_ps[:],
                                    axis=mybir.AxisListType.X,
                                    op=mybir.AluOpType.add,
                                )
                                nc.vector.tensor_add(
                                    rsum_sb[0:64, :], rsraw_sb[:], cor64_sb[:]
                                )
                                # odd heads' rowsums also needed at
                                # partitions 64:127 for the bias matvec
                                nc.sync.dma_start(
                                    rsum_sb[64:128, :], rsum_sb[0:64, :]
                                )

                            def _mk_hook_m(c0):
                                def _hook():
                                    for c in (c0, c0 + 1):
                                        pm = pbig.tile([P, D], dt.float32, tag="pbig")
                                        for n in range(2):
                                            nc.tensor.matmul(
                                                pm[:, n * 512 : (n + 1) * 512],
                                                lhsT=kv8[:, c, :, :],
                                                rhs=wo2_sb[:, 2 * c : 2 * c + 2, n * 512 : (n + 1) * 512],
                                                start=True,
                                                stop=True,
                                                perf_mode=PM.DoubleRow,
                                            )
                                        if c % 2 == 0:
                                            nc.scalar.copy(
                                                out=m8[:, c, :], in_=pm[:]
                                            )
                                        else:
                                            nc.vector.tensor_copy(
                                                out=m8[:, c, :], in_=pm[:]
                                            )
                                return _hook

                            def _hook_bias():
                                bt = pbig.tile([P, D], dt.float32, tag="pbig")
                                nc.tensor.matmul(
                                    bt[:, 0:8], lhsT=zz[:1, :P], rhs=zz[:1, P : P + 8],
                                    start=True, stop=True, skip_group_check=True,
                                )
                                for oc in range(DC):
                                    for c in range(DC):
                                        for s in range(2):
                                            pr = slice(64 * s, 64 * s + 64)
                                            nc.tensor.matmul(
                                                bt[:, oc : oc + 1],
                                                lhsT=wob_sb[pr, c, oc * P : (oc + 1) * P],
                                                rhs=rsum_sb[pr, 2 * c + s : 2 * c + s + 1],
                                                start=False,
                                                stop=(oc == DC - 1 and c == DC - 1 and s == 1),
                                                skip_group_check=True,
                                            )
                                nc.vector.tensor_add(
                                    bias_sb[:], bt[:, 0:8], bo_sb[:]
                                )

                            hooks = {
                                0: _hook_flush,
                                1: _hook_repack,
                                2: _mk_hook_m(0),
                                3: _mk_hook_m(2),
                                4: _mk_hook_m(4),
                                5: _mk_hook_m(6),
                                7: _hook_bias,
                            }
                            _q_section(pair, xts, hooks)
                        else:
                            _q_section(pair, xts)
                            _kv_section(pair, xts)

                    if kv_pend[0] is not None:
                        _emit_kv(kv_pend[0], True)
                        kv_pend[0] = None

                    # ---- phase 2: yT = m8.T @ dq + bias ----
                    # kp rotated per tile so the first tiles only need the
                    # early m8 chunks (drained during the q hooks).
                    for oc in range(DC):
                        for qb in range(4):
                            py = pbig.tile([P, 2 * TT], dt.float32, tag="pbig")
                            for half in range(2):
                                for i in range(NKP):
                                    kp = (oc * 4 + qb + i) % NKP
                                    nc.tensor.matmul(
                                        py[:, half * TT : (half + 1) * TT],
                                        lhsT=m8[:, 2 * kp : 2 * kp + 2, oc * P : (oc + 1) * P],
                                        rhs=phi_q[:, 2 * kp : 2 * kp + 2,
                                                  qb * 1024 + half * TT : qb * 1024 + (half + 1) * TT],
                                        start=(i == 0),
                                        stop=(i == NKP - 1),
                                        perf_mode=PM.DoubleRow,
                                    )
                            ys0 = ypool.tile([P, TT], dt.bfloat16, tag="ys0")
                            nc.scalar.activation(
                                ys0[:], py[:, 0:TT], AF.Identity,
                                bias=bias_sb[:, oc : oc + 1], scale=1.0,
                            )
                            nc.sync.dma_start(
                                yT_d[oc * P : (oc + 1) * P, qb * 1024 : qb * 1024 + TT],
                                ys0[:],
                            )
                            ys1 = ypool.tile([P, TT], dt.bfloat16, tag="ys1")
                            nc.vector.tensor_scalar_add(
                                ys1[:], py[:, TT : 2 * TT], bias_sb[:, oc : oc + 1]
                            )
                            nc.gpsimd.dma_start(
                                yT_d[oc * P : (oc + 1) * P, qb * 1024 + TT : (qb + 1) * 1024],
                                ys1[:],
                            )
    _split_multi_waits(nc)
    return nc


def _get_program():
    key = "nc"
    if key not in _CACHE:
        _CACHE[key] = _build_program()
    return _CACHE[key]


def _q8(a):
    return np.clip(np.asarray(a, np.float32), -240.0, 240.0).astype(_F8)


def _prep_shared(Wq, bq, Wk, Wv, bv, Wo, bo):
    woT = np.ascontiguousarray(Wo.T)  # [e, o]
    shared = {
        "wq": _q8(np.ascontiguousarray(Wq.T)),
        "wk": _q8(np.ascontiguousarray(Wk.T)),
        "wv": _q8(np.ascontiguousarray(Wv.T)),
        "wo2": _q8(8.0 * woT.reshape(H, 64, D).transpose(1, 0, 2)),
        "wob": np.ascontiguousarray(woT).astype(_BF16),
        "bqs": np.ascontiguousarray(
            (SCALE * bq).astype(np.float32).reshape(DC, P).T
        ),
        "bos": np.ascontiguousarray(bo.astype(np.float32).reshape(DC, P).T),
        "bvb": np.ascontiguousarray(
            np.broadcast_to(bv.astype(np.float32), (P, D))
        ),
    }
    return shared


def _run(in_maps, **kw):
    from concourse.bass_utils import run_bass_kernel_spmd

    nc = _get_program()
    return run_bass_kernel_spmd(nc, in_maps, list(range(NCORES)), **kw)


def kernel(x, Wq, bq, Wk, Wv, bv, Wo, bo):
    x = np.asarray(x, dtype=np.float32)
    assert x.shape == (B, T, D), x.shape
    Wv = np.asarray(Wv, np.float32)
    bv = np.asarray(bv, np.float32)
    shared = _prep_shared(
        np.asarray(Wq, np.float32), np.asarray(bq, np.float32),
        np.asarray(Wk, np.float32), Wv, bv,
        np.asarray(Wo, np.float32), np.asarray(bo, np.float32),
    )
    in_maps = []
    for b in range(B):
        m = dict(shared)
        m["xT"] = _q8(x[b].T)
        # exact rank-1 kv correction: S_v[e] = sum_t v_exact[t, e]
        sv = (x[b].astype(np.float64).sum(0) @ Wv.T.astype(np.float64)
              + T * bv.astype(np.float64)).astype(np.float32)
        cor = np.ascontiguousarray(sv.reshape(H, 64).T)  # [64, H]
        m["cor1"] = cor
        m["cor64"] = 64.0 * cor
        in_maps.append(m)

    res = _run(in_maps)
    out = np.empty((B, T, D), np.float32)
    for b in range(B):
        out[b] = res.results[b]["yT"].astype(np.float32).T
    return out


# revision 9
# speedup vs baseline: 3.3256x; 1.0719x over previous
"""Linear attention (silu+1 feature map) MultiHeadAttention kernel for 8x TRN2.

Sharding: data-parallel over batch (B=8 -> 1 batch element per NeuronCore).

fp8 DoubleRow design. All four big GEMMs run as fp8e4 DoubleRow matmuls
(contraction 256 per instruction at 0.5 cycles/output-row). Accuracy is
preserved by a mean-split: phi(x) = silu(x)+1 = 1 + d(x), so every fp8
operand that multiplies a full-magnitude tensor is a small delta:

  phase 1 (token tiles of 512, processed in pairs):
    qT[o,t]  = wq8.T @ x8T            (DoubleRow, feature-major out)
    dq[o,t]  = silu(s*qT + s*bq)      (ACT, fp8 out; phi_q = 1 + dq)
    k[t,o]   = x8T.T @ wk8            (DoubleRow, token-major)
    v[t,e]   = x8T.T @ wv8 + bv       (DoubleRow + DVE bias add, fp8 out)
    dk[t,o]  = silu(s*k)              (ACT, fp8 out; phi_k = 1 + dk)
    kv[e,d]_h += v_h.T @ dk_h         (DoubleRow over token sub-tile pairs,
                                       [64,1024] psum: head h at cols 64h)
  corrections (the "1" parts of phi_k):
    kv_full  = kv + S_v[e]            S_v = sum_t v_exact[t] computed on host
                                      from the original fp32 x (rank-1 term)
  M stage:
    kv8dr    = block-diag fp8 repack of kv_full  ([64, 2, 128] per chunk)
    M[d,o]_c = kv8dr_c .T@ wo28_pair  (DoubleRow, contraction = 2 heads' e)
    m8       = fp8(M)
  colsum (the "1" part of phi_q): y += colsum(M) as a per-partition bias:
    rowsum_kv[e] = reduce_d kv_full   (DVE reduce + corr64)
    bias[o]  = sum_e rowsum_kv[e] wo[e,o] + bo   (128 N=1 matvecs, bf16)
  phase 2:
    yT[o,t]  = m8.T @ dq + bias       (DoubleRow; bias folded into drains)

Host side: fp8 casts (clip +-240), S_v correction, output upcast bf16->fp32.
"""

import numpy as np
import ml_dtypes

B, T, D = 8, 4096, 1024
H, DH = 16, 64
SCALE = float(DH ** -0.25)
NCORES = 8
P = 128
DC = D // P          # 8 feature chunks
NKP = DC // 2        # 4 k-pairs for DoubleRow
TT = 512             # token tile (phase 1)
NTT = T // TT        # 8 token tiles
NSUB = TT // P       # 4 sub-tiles of 128 tokens

_BF16 = ml_dtypes.bfloat16
_F8 = ml_dtypes.float8_e4m3fn

_CACHE = {}


def _split_multi_waits(nc):
    """walrus in this container only encodes ONE sync-wait command per
    instruction. Hoist extra waits onto injected same-engine NOPs placed
    immediately before the instruction (program order on the engine queue
    makes this semantically identical)."""
    import concourse.mybir as mybir

    n_split = 0
    for fn in nc.m.functions:
        for bb in fn.blocks:
            new = []
            changed = False
            for inst in bb.instructions:
                si = inst.sync_info
                waits = list(si.on_wait) if si is not None else []
                if len(waits) > 1:
                    changed = True
                    for j, w in enumerate(waits[:-1]):
                        nop = mybir.InstNoOp(
                            name=f"{inst.name}-sw{j}", ins=[], outs=[]
                        )
                        nop.engine = inst.engine
                        nop.sync_info = mybir.SyncInfo(
                            on_wait=[w], on_update=[]
                        )
                        new.append(nop)
                        n_split += 1
                    inst.sync_info = mybir.SyncInfo(
                        on_wait=[waits[-1]], on_update=list(si.on_update)
                    )
                new.append(inst)
            if changed:
                bb.instructions = new
    return n_split


def _build_program():
    import concourse.bass as bass
    import concourse.mybir as mybir
    from concourse.tile import TileContext, add_dep_helper

    dt = mybir.dt
    AF = mybir.ActivationFunctionType
    PM = mybir.MatmulPerfMode

    nc = bass.Bass()

    xT_d = nc.dram_tensor("xT", [D, T], dt.float8e4, kind="ExternalInput")
    wq_d = nc.dram_tensor("wq", [D, D], dt.float8e4, kind="ExternalInput")
    wk_d = nc.dram_tensor("wk", [D, D], dt.float8e4, kind="ExternalInput")
    wv_d = nc.dram_tensor("wv", [D, D], dt.float8e4, kind="ExternalInput")
    wo2_d = nc.dram_tensor("wo2", [64, H, D], dt.float8e4, kind="ExternalInput")
    wob_d = nc.dram_tensor("wob", [D, D], dt.bfloat16, kind="ExternalInput")
    bqs_d = nc.dram_tensor("bqs", [P, DC], dt.float32, kind="ExternalInput")
    bos_d = nc.dram_tensor("bos", [P, DC], dt.float32, kind="ExternalInput")
    bvb_d = nc.dram_tensor("bvb", [P, D], dt.float32, kind="ExternalInput")
    cor1_d = nc.dram_tensor("cor1", [64, H], dt.float32, kind="ExternalInput")
    cor64_d = nc.dram_tensor("cor64", [64, H], dt.float32, kind="ExternalInput")
    yT_d = nc.dram_tensor("yT", [D, T], dt.bfloat16, kind="ExternalOutput")

    with TileContext(nc) as tc:
        with (
            tc.tile_pool(name="weights", bufs=1) as wpool,
            tc.tile_pool(name="phiq", bufs=1) as qpool,
            tc.tile_pool(name="msb", bufs=1) as mpool,
            tc.tile_pool(name="xin", bufs=3) as xpool,
            tc.tile_pool(name="kvtiles", bufs=4) as kvpool,
            tc.tile_pool(name="yout", bufs=3) as ypool,
        ):
            # ---- weight / const preload ----
            wq_sb = wpool.tile([P, DC, D], dt.float8e4, tag="wq")
            wk_sb = wpool.tile([P, DC, D], dt.float8e4, tag="wk")
            wv_sb = wpool.tile([P, DC, D], dt.float8e4, tag="wv")
            wo2_sb = wpool.tile([64, H, D], dt.float8e4, tag="wo2")
            wob_sb = wpool.tile([P, DC, D], dt.bfloat16, tag="wob")
            bq_sb = wpool.tile([P, DC], dt.float32, tag="bq")
            bo_sb = wpool.tile([P, DC], dt.float32, tag="bo")
            bv_sb = wpool.tile([P, D], dt.float32, tag="bv")
            cor1_sb = wpool.tile([64, H], dt.float32, tag="cor1")
            cor64_sb = wpool.tile([64, H], dt.float32, tag="cor64")
            bias_sb = wpool.tile([P, DC], dt.float32, tag="bias")
            rsum_sb = wpool.tile([P, H], dt.bfloat16, tag="rsum")
            rsraw_sb = wpool.tile([64, H], dt.float32, tag="rsraw")

            wq_r = wq_d.rearrange("(c p) o -> p c o", p=P)
            xT_r = xT_d.rearrange("(c p) t -> p c t", p=P)

            # first q matmuls gate on the pair-0 x slab + first wq column
            # half (covers oc 0-3); x pairs 1-3 stream on the gpsimd queue.
            xt01 = xpool.tile([P, DC, 2 * TT], dt.float8e4, tag="xt0", bufs=1)
            nc.sync.dma_start(wq_sb[:, :, 0:512], wq_r[:, :, 0:512])
            nc.sync.dma_start(xt01[:], xT_r[:, :, 0 : 2 * TT])
            nc.sync.dma_start(wq_sb[:, :, 512:1024], wq_r[:, :, 512:1024])
            xt_pre = [xt01[:, :, 0:TT], xt01[:, :, TT : 2 * TT]]
            nc.sync.dma_start(bq_sb[:], bqs_d[:])
            nc.sync.dma_start(wk_sb[:], wk_d.rearrange("(c p) o -> p c o", p=P))
            nc.sync.dma_start(bv_sb[:], bvb_d[:])
            nc.sync.dma_start(wv_sb[:], wv_d.rearrange("(c p) o -> p c o", p=P))
            nc.sync.dma_start(bo_sb[:], bos_d[:])
            nc.sync.dma_start(cor1_sb[:], cor1_d[:])
            nc.sync.dma_start(cor64_sb[:], cor64_d[:])
            nc.sync.dma_start(wo2_sb[:], wo2_d[:])
            nc.sync.dma_start(wob_sb[:], wob_d.rearrange("(c p) o -> p c o", p=P))

            phi_q = qpool.tile([P, DC, T], dt.float8e4, tag="phiq")
            m8 = mpool.tile([P, DC, D], dt.float8e4, tag="m8")
            # block-diag fp8 repack of kv: chunk c = [64, 2, 128]; j=0 holds
            # head 2c in cols 0:64, j=1 holds head 2c+1 in cols 64:128, rest 0
            kv8 = mpool.tile([64, DC, 2, P], dt.float8e4, tag="kv8")

            zz = wpool.tile([1, 640], dt.bfloat16, tag="zz")
            nc.vector.memset(zz[:], 0.0)
            nc.gpsimd.memset(kv8[:], 0.0)

            with tc.tile_pool(name="ps_kv", bufs=1, space="PSUM") as pkv_pool:
                # kv accumulator: head h in columns [64h, 64h+64), rows = e
                kv_ps = pkv_pool.tile([64, H, 64], dt.float32, tag="kvacc")

                def _zero_kv(tag):
                    for i in range(2):
                        nc.tensor.matmul(
                            kv_ps[:, 8 * i : 8 * i + 8, :],
                            lhsT=zz[:1, :64], rhs=zz[:1, 64:576],
                            start=True, stop=True, skip_group_check=True,
                        )

                _zero_kv("z0")
                # warmup matmuls filling the startup DMA shadow: keep the PE
                # p-state warm so the first real matmuls run at full clock.
                for w in range(40):
                    nc.tensor.matmul(
                        kv_ps[:, 0:2, :], lhsT=zz[:1, :64], rhs=zz[:1, 64:192],
                        start=True, stop=True, skip_group_check=True,
                    )
                _zero_kv("z1")

                kv_pend = [None]

                def _emit_kv(pending, last):
                    v2_p, dk2_p = pending
                    for h in range(H):
                        nc.tensor.matmul(
                            kv_ps[:, h, :],
                            lhsT=v2_p[:, :, h * 64 : (h + 1) * 64],
                            rhs=dk2_p[:, :, h * 64 : (h + 1) * 64],
                            start=False,
                            stop=last and h == H - 1,
                            skip_group_check=True,
                            perf_mode=PM.DoubleRow,
                        )

                with tc.tile_pool(name="ps_big", bufs=3, space="PSUM") as pbig:

                    # ---- q projection (feature-major out). One [128,1024]
                    # psum per oc covers both tiles of the pair. ----
                    def _q_section(pair, xts, post_oc=None):
                        for oc in range(DC):
                            pq = pbig.tile([P, 2 * TT], dt.float32, tag="pbig")
                            for half in range(2):
                                for kp in range(NKP):
                                    nc.tensor.matmul(
                                        pq[:, half * TT : (half + 1) * TT],
                                        lhsT=wq_sb[:, 2 * kp : 2 * kp + 2, oc * P : (oc + 1) * P],
                                        rhs=xts[half][:, 2 * kp : 2 * kp + 2, :],
                                        start=(kp == 0),
                                        stop=(kp == NKP - 1),
                                        perf_mode=PM.DoubleRow,
                                    )
                            nc.scalar.activation(
                                phi_q[:, oc, pair * 2 * TT : (pair + 1) * 2 * TT],
                                pq[:], AF.Silu,
                                bias=bq_sb[:, oc : oc + 1], scale=SCALE,
                            )
                            if post_oc is not None and oc in post_oc:
                                post_oc[oc]()

                    # ---- k,v projections + kv accumulation (DoubleRow over
                    # sub-tile pairs; kv matmuls emitted one sub-pair late) ----
                    def _kv_section(pair, xts):
                        v2 = dk2 = None
                        for half in range(2):
                            xt = xts[half]
                            for sub in range(NSUB):
                                g = half * NSUB + sub
                                j = g % 2
                                xs = xt[:, :, sub * P : (sub + 1) * P]
                                pk = pbig.tile([P, D], dt.float32, tag="pbig")
                                pv = pbig.tile([P, D], dt.float32, tag="pbig")
                                for n in range(2):
                                    for kp in range(NKP):
                                        nc.tensor.matmul(
                                            pk[:, n * 512 : (n + 1) * 512],
                                            lhsT=xs[:, 2 * kp : 2 * kp + 2, :],
                                            rhs=wk_sb[:, 2 * kp : 2 * kp + 2, n * 512 : (n + 1) * 512],
                                            start=(kp == 0),
                                            stop=(kp == NKP - 1),
                                            perf_mode=PM.DoubleRow,
                                        )
                                for n in range(2):
                                    for kp in range(NKP):
                                        nc.tensor.matmul(
                                            pv[:, n * 512 : (n + 1) * 512],
                                            lhsT=xs[:, 2 * kp : 2 * kp + 2, :],
                                            rhs=wv_sb[:, 2 * kp : 2 * kp + 2, n * 512 : (n + 1) * 512],
                                            start=(kp == 0),
                                            stop=(kp == NKP - 1),
                                            perf_mode=PM.DoubleRow,
                                        )
                                if j == 0:
                                    v2 = kvpool.tile([P, 2, D], dt.float8e4, tag="v2")
                                    dk2 = kvpool.tile([P, 2, D], dt.float8e4, tag="dk2")
                                nc.scalar.activation(
                                    dk2[:, j, :], pk[:], AF.Silu, scale=SCALE
                                )
                                nc.vector.tensor_add(v2[:, j, :], pv[:], bv_sb[:])
                                if j == 1:
                                    if kv_pend[0] is not None:
                                        _emit_kv(kv_pend[0], False)
                                    kv_pend[0] = (v2, dk2)

                    for pair in range(NTT // 2):
                        if pair == 0:
                            xts = xt_pre
                        else:
                            xts = []
                            for half in range(2):
                                tt = pair * 2 + half
                                xt = xpool.tile([P, DC, TT], dt.float8e4, tag="xt")
                                nc.gpsimd.dma_start(
                                    xt[:], xT_r[:, :, tt * TT : (tt + 1) * TT]
                                )
                                xts.append(xt)

                        if pair == NTT // 2 - 1:
                            # last pair: kv section first; the kv flush,
                            # repack, M stage and colsum spread across the q
                            # chunk boundaries so they hide under q matmuls.
                            _kv_section(pair, xts)

                            def _hook_flush():
                                _emit_kv(kv_pend[0], True)
                                kv_pend[0] = None

                            def _hook_repack():
                                # kv can exceed fp8e4's +-240: store kv/8 in
                                # fp8 (wo2 is pre-scaled by 8 on the host)
                                for c in range(DC):
                                    nc.vector.tensor_scalar(
                                        kv8[:, c, 0, 0:64],
                                        kv_ps[:, 2 * c, :],
                                        cor1_sb[:, 2 * c : 2 * c + 1],
                                        0.125,
                                        op0=mybir.AluOpType.add,
                                        op1=mybir.AluOpType.mult,
                                    )
                                    nc.vector.tensor_scalar(
                                        kv8[:, c, 1, 64:128],
                                        kv_ps[:, 2 * c + 1, :],
                                        cor1_sb[:, 2 * c + 1 : 2 * c + 2],
                                        0.125,
                                        op0=mybir.AluOpType.add,
                                        op1=mybir.AluOpType.mult,
                                    )
                                nc.vector.tensor_reduce(
                                    rsraw_sb[:], kv_ps[:],
                                    axis=mybir.AxisListType.X,
                                    op=mybir.AluOpType.add,
                                )
                                nc.vector.tensor_add(
                                    rsum_sb[0:64, :], rsraw_sb[:], cor64_sb[:]
                                )
                                # odd heads' rowsums also needed at
                                # partitions 64:127 for the bias matvec
                                nc.sync.dma_start(
                                    rsum_sb[64:128, :], rsum_sb[0:64, :]
                                )

                            def _mk_hook_m(c0):
                                def _hook():
                                    for c in (c0, c0 + 1):
                                        pm = pbig.tile([P, D], dt.float32, tag="pbig")
                                        for n in range(2):
                                            nc.tensor.matmul(
                                                pm[:, n * 512 : (n + 1) * 512],
                                                lhsT=kv8[:, c, :, :],
                                                rhs=wo2_sb[:, 2 * c : 2 * c + 2, n * 512 : (n + 1) * 512],
                                                start=True,
                                                stop=True,
                                                perf_mode=PM.DoubleRow,
                                            )
                                        if c % 2 == 0:
                                            nc.scalar.copy(
                                                out=m8[:, c, :], in_=pm[:]
                                            )
                                        else:
                                            nc.vector.tensor_copy(
                                                out=m8[:, c, :], in_=pm[:]
                                            )
                                return _hook

                            def _hook_bias():
                                bt = pbig.tile([P, D], dt.float32, tag="pbig")
                                nc.tensor.matmul(
                                    bt[:, 0:8], lhsT=zz[:1, :P], rhs=zz[:1, P : P + 8],
                                    start=True, stop=True, skip_group_check=True,
                                )
                                for oc in range(DC):
                                    for c in range(DC):
                                        for s in range(2):
                                            pr = slice(64 * s, 64 * s + 64)
                                            nc.tensor.matmul(
                                                bt[:, oc : oc + 1],
                                                lhsT=wob_sb[pr, c, oc * P : (oc + 1) * P],
                                                rhs=rsum_sb[pr, 2 * c + s : 2 * c + s + 1],
                                                start=False,
                                                stop=(oc == DC - 1 and c == DC - 1 and s == 1),
                                                skip_group_check=True,
                                            )
                                nc.vector.tensor_add(
                                    bias_sb[:], bt[:, 0:8], bo_sb[:]
                                )

                            hooks = {
                                0: _hook_flush,
                                1: _hook_repack,
                                2: _mk_hook_m(0),
                                3: _mk_hook_m(2),
                                4: _mk_hook_m(4),
                                5: _mk_hook_m(6),
                                7: _hook_bias,
                            }
                            _q_section(pair, xts, hooks)
                        else:
                            _q_section(pair, xts)
                            _kv_section(pair, xts)

                    if kv_pend[0] is not None:
                        _emit_kv(kv_pend[0], True)
                        kv_pend[0] = None

                    # ---- phase 2: yT = m8.T @ dq + bias ----
                    # kp rotated per tile so the first tiles only need the
                    # early m8 chunks (drained during the q hooks). Two qb
                    # tiles share one [P, 2048] output buffer -> half the
                    # DMAs at double the size; queues alternate sync/gpsimd.
                    # The last tile drains in 512-col pieces on ACT+DVE with
                    # its own small DMAs so the kernel tail stays short.
                    nt2 = 0
                    for oc in range(DC):
                        for qp in range(2):
                            last_pair = oc == DC - 1 and qp == 1
                            ysb = ypool.tile([P, 4 * TT], dt.bfloat16, tag="ysb")
                            for qh in range(2):
                                qb = qp * 2 + qh
                                py = pbig.tile([P, 2 * TT], dt.float32, tag="pbig")
                                for half in range(2):
                                    for i in range(NKP):
                                        kp = (nt2 + i) % NKP
                                        nc.tensor.matmul(
                                            py[:, half * TT : (half + 1) * TT],
                                            lhsT=m8[:, 2 * kp : 2 * kp + 2, oc * P : (oc + 1) * P],
                                            rhs=phi_q[:, 2 * kp : 2 * kp + 2,
                                                      qb * 1024 + half * TT : qb * 1024 + (half + 1) * TT],
                                            start=(i == 0),
                                            stop=(i == NKP - 1),
                                            perf_mode=PM.DoubleRow,
                                        )
                                nt2 += 1
                                if last_pair and qh == 1:
                                    # final tile: split drains, parallel DMAs
                                    nc.scalar.activation(
                                        ysb[:, 2 * TT : 3 * TT], py[:, 0:TT],
                                        AF.Identity,
                                        bias=bias_sb[:, oc : oc + 1], scale=1.0,
                                    )
                                    nc.sync.dma_start(
                                        yT_d[oc * P : (oc + 1) * P,
                                             qb * 1024 : qb * 1024 + TT],
                                        ysb[:, 2 * TT : 3 * TT],
                                    )
                                    nc.vector.tensor_scalar_add(
                                        ysb[:, 3 * TT : 4 * TT], py[:, TT : 2 * TT],
                                        bias_sb[:, oc : oc + 1],
                                    )
                                    nc.gpsimd.dma_start(
                                        yT_d[oc * P : (oc + 1) * P,
                                             qb * 1024 + TT : (qb + 1) * 1024],
                                        ysb[:, 3 * TT : 4 * TT],
                                    )
                                elif qh == 0:
                                    nc.scalar.activation(
                                        ysb[:, 0 : 2 * TT], py[:],
                                        AF.Identity,
                                        bias=bias_sb[:, oc : oc + 1], scale=1.0,
                                    )
                                else:
                                    nc.vector.tensor_scalar_add(
                                        ysb[:, 2 * TT : 4 * TT], py[:],
                                        bias_sb[:, oc : oc + 1],
                                    )
                            if last_pair:
                                nc.gpsimd.dma_start(
                                    yT_d[oc * P : (oc + 1) * P,
                                         qp * 2048 : qp * 2048 + 2 * TT],
                                    ysb[:, 0 : 2 * TT],
                                )
                            elif (oc * 2 + qp) % 2 == 0:
                                nc.sync.dma_start(
                                    yT_d[oc * P : (oc + 1) * P,
                                         qp * 2048 : (qp + 1) * 2048],
                                    ysb[:],
                                )
                            else:
                                nc.gpsimd.dma_start(
                                    yT_d[oc * P : (oc + 1) * P,
                                         qp * 2048 : (qp + 1) * 2048],
                                    ysb[:],
                                )
    _split_multi_waits(nc)
    return nc


def _get_program():
    key = "nc"
    if key not in _CACHE:
        _CACHE[key] = _build_program()
    return _CACHE[key]


def _q8(a):
    return np.clip(np.asarray(a, np.float32), -240.0, 240.0).astype(_F8)


def _prep_shared(Wq, bq, Wk, Wv, bv, Wo, bo):
    woT = np.ascontiguousarray(Wo.T)  # [e, o]
    shared = {
        "wq": _q8(np.ascontiguousarray(Wq.T)),
        "wk": _q8(np.ascontiguousarray(Wk.T)),
        "wv": _q8(np.ascontiguousarray(Wv.T)),
        "wo2": _q8(8.0 * woT.reshape(H, 64, D).transpose(1, 0, 2)),
        "wob": np.ascontiguousarray(woT).astype(_BF16),
        "bqs": np.ascontiguousarray(
            (SCALE * bq).astype(np.float32).reshape(DC, P).T
        ),
        "bos": np.ascontiguousarray(bo.astype(np.float32).reshape(DC, P).T),
        "bvb": np.ascontiguousarray(
            np.broadcast_to(bv.astype(np.float32), (P, D))
        ),
    }
    return shared


def _run(in_maps, **kw):
    from concourse.bass_utils import run_bass_kernel_spmd

    nc = _get_program()
    return run_bass_kernel_spmd(nc, in_maps, list(range(NCORES)), **kw)


def kernel(x, Wq, bq, Wk, Wv, bv, Wo, bo):
    x = np.asarray(x, dtype=np.float32)
    assert x.shape == (B, T, D), x.shape
    Wv = np.asarray(Wv, np.float32)
    bv = np.asarray(bv, np.float32)
    shared = _prep_shared(
        np.asarray(Wq, np.float32), np.asarray(bq, np.float32),
        np.asarray(Wk, np.float32), Wv, bv,
        np.asarray(Wo, np.float32), np.asarray(bo, np.float32),
    )
    in_maps = []
    for b in range(B):
        m = dict(shared)
        m["xT"] = _q8(x[b].T)
        # exact rank-1 kv correction: S_v[e] = sum_t v_exact[t, e]
        sv = (x[b].astype(np.float64).sum(0) @ Wv.T.astype(np.float64)
              + T * bv.astype(np.float64)).astype(np.float32)
        cor = np.ascontiguousarray(sv.reshape(H, 64).T)  # [64, H]
        m["cor1"] = cor
        m["cor64"] = 64.0 * cor
        in_maps.append(m)

    res = _run(in_maps)
    out = np.empty((B, T, D), np.float32)
    for b in range(B):
        out[b] = res.results[b]["yT"].astype(np.float32).T
    return out
